# revision 9
# baseline (speedup 1.0000x reference)
"""Trainium2 kernel for diamond-search block motion estimation + compensation.

v2 strategy (vs v1 brute-force 17x17 volume):
- The LDSP stage of diamond search only ever evaluates positions with
  dy+dx even (all LDSP moves preserve parity, starting from (0,0)).  So the
  device computes the 17dy x 9dx even-parity half-volume only; the final
  SDSP refinement needs <=4 odd-parity costs per block at data-dependent
  positions, computed on host (same nature as the host walk itself).
- Per unit = (frame pair, 128-row chunk): 240 units, 30 per core.
  For each dy: the dy-shifted reference rows are produced by a 0/1
  shift-matmul on the (otherwise idle) TensorE into PSUM -- bit-exact fp32
  pass-through, eliminating the 17x-replicated HBM DMA of v1.  DVE (or
  GPSIMD for a subset of dy, for engine balance) computes P - shift(I) for
  the 9 parity dx windows in one strided op; a single DVE
  tensor_reduce(apply_absolute_value=True) fuses abs + the 8-column SAD
  sum (v1 used a scalar-engine abs pass + 3 levels of DVE tree adds);
  TensorE reduces the 8 rows of each block row via a 0/1 selector matmul.
- Host: exact diamond-search walk on the cost tables (vectorized numpy
  with analytic validity masks reproducing the reference's LARGE-cost
  rules), host SDSP refinement, then block compensation and cropping.

The walk compares cost *sums*; the reference compares means (sum/64) -- an
exact power-of-two scaling, so argmin decisions are identical.
"""
import numpy as np
from contextlib import ExitStack

import concourse.bass as bass
import concourse.bacc as bacc
import concourse.mybir as mybir
import concourse.tile as tile
from concourse.bass_utils import run_bass_kernel_spmd

MB = 8
P = 8
CROP = 17
LARGE_SUM = np.float32(65537.0 * 64)
MAX_STEPS = 16
# (dx, dy) pairs, order matters for argmin tie-breaks (matches reference)
LDSP = np.array([[0, -2], [-1, -1], [1, -1], [-2, 0], [0, 0], [2, 0],
                 [-1, 1], [1, 1], [0, 2]], dtype=np.int32)
SDSP = np.array([[0, -1], [-1, 0], [0, 0], [1, 0], [0, 1]], dtype=np.int32)

B, T, H, W = 4, 16, 512, 512
NBR, NBC = H // MB, W // MB          # 64 x 64 blocks
NPAIR = B * (T - 1)                  # 60 motion fields
CHUNKS = 4                           # row chunks of 128
NUNIT = NPAIR * CHUNKS               # 240 units
NCORES = 8
UPC = NUNIT // NCORES                # 30 units per core, exact
NWIN = 9                             # parity dx windows per dy
WI = 536                             # padded I width: 8 + 512 + 8 + 8
# dy iterations whose subtract runs on GPSIMD (engine balance: DVE does all
# 17 abs-reduces + 6 subs; GPSIMD does 11 subs -- the wider even-dy ones)
GP_SET = {0, 2, 4, 6, 8, 10, 12, 14, 16, 5, 11}

_CACHED_NC = None


def _build_nc(nproc=UPC, static=True, repeat=1, gp_set=None):
    """Device program: per unit, 17dy x 9dx even-parity cost volume for 16
    block rows.  I/O shapes fixed; nproc/static/repeat for differential
    timing (same contract as v1)."""
    gp_set = GP_SET if gp_set is None else gp_set
    nc = bacc.Bacc()
    f32 = mybir.dt.float32
    xP = nc.dram_tensor("xP", [UPC * 128, 512], f32, kind="ExternalInput")
    xI = nc.dram_tensor("xI", [UPC * 144, WI], f32, kind="ExternalInput")
    sall = nc.dram_tensor("sall", [128, 17 * 128], f32, kind="ExternalInput")
    shi = nc.dram_tensor("shi", [16, 17 * 128], f32, kind="ExternalInput")
    sel = nc.dram_tensor("sel", [128, 16], f32, kind="ExternalInput")
    vol = nc.dram_tensor("vol", [UPC * 17 * 16, NWIN * 64], f32,
                         kind="ExternalOutput")

    with tile.TileContext(nc) as tc, ExitStack() as ctx:
        cpool = ctx.enter_context(tc.tile_pool(name="cpool", bufs=1))
        upool = ctx.enter_context(tc.tile_pool(name="upool", bufs=2))
        wpool = ctx.enter_context(tc.tile_pool(name="wpool", bufs=2))
        opool = ctx.enter_context(tc.tile_pool(name="opool", bufs=2))
        psA = ctx.enter_context(tc.tile_pool(name="psA", bufs=2, space="PSUM"))
        psB = ctx.enter_context(tc.tile_pool(name="psB", bufs=2, space="PSUM"))

        sal_t = cpool.tile([128, 17 * 128], f32, tag="sall")
        shi_t = cpool.tile([16, 17 * 128], f32, tag="shi")
        sel_t = cpool.tile([128, 16], f32, tag="sel")
        nc.sync.dma_start(sal_t[:, :], sall[:, :])
        nc.sync.dma_start(shi_t[:, :], shi[:, :])
        nc.sync.dma_start(sel_t[:, :], sel[:, :])

        def unit_body(u):
            p_t = upool.tile([128, 512], f32, tag="p")
            ilo = upool.tile([128, WI], f32, tag="ilo")
            ihi = upool.tile([16, WI], f32, tag="ihi")
            nc.sync.dma_start(p_t[:, :], xP[bass.ts(u, 128), :])
            nc.sync.dma_start(ilo[:, :], xI[bass.ds(u * 144, 128), :])
            nc.sync.dma_start(ihi[:, :], xI[bass.ds(u * 144 + 128, 16), :])

            for dyi in range(17):
                st = sal_t[:, dyi * 128:(dyi + 1) * 128]
                ish = psA.tile([128, WI], f32, tag="ish")
                # bottom partitions read I_unit row p+ny+8 up to 135+ny;
                # only dyi=0 (ny=-8) stays entirely within I_lo
                need_hi = dyi > 0
                for c0, c1 in ((0, 512), (512, WI)):
                    nc.tensor.matmul(ish[:, c0:c1], st, ilo[:, c0:c1],
                                     start=True, stop=not need_hi)
                    if need_hi:
                        st_hi = shi_t[:, dyi * 128:(dyi + 1) * 128]
                        nc.tensor.matmul(ish[:, c0:c1], st_hi, ihi[:, c0:c1],
                                         start=False, stop=True)

                parity = dyi & 1
                nw = 8 if parity else 9  # odd dy: only 8 valid dx windows
                d_t = wpool.tile([128, NWIN, 512], f32, tag="d")
                in0 = p_t[:, :].unsqueeze(1).broadcast_to([128, nw, 512])
                if dyi in gp_set:
                    # GPSIMD has no PSUM port: stage shift through SBUF
                    ish_sb = wpool.tile([128, WI], f32, tag="ishsb")
                    nc.scalar.copy(ish_sb[:, :], ish[:, :])
                    iv = ish_sb[:, :]
                    eng = nc.gpsimd
                else:
                    iv = ish[:, :]
                    eng = nc.vector
                in1 = bass.AP(iv.tensor, offset=iv.offset + parity,
                              ap=[iv.ap[0], [2, nw], [1, 512]])
                eng.tensor_sub(d_t[:, :nw, :], in0, in1)

                rc = wpool.tile([128, NWIN * 64], f32, tag="rc")
                dv = d_t[:, :, :]
                din = bass.AP(dv.tensor, offset=dv.offset,
                              ap=[dv.ap[0], [512, nw], [8, 64], [1, 8]])
                rv = rc[:, :]
                rout = bass.AP(rv.tensor, offset=rv.offset,
                               ap=[rv.ap[0], [64, nw], [1, 64]])
                nc.vector.tensor_reduce(rout, din, axis=mybir.AxisListType.X,
                                        op=mybir.AluOpType.add,
                                        apply_absolute_value=True)

                ps = psB.tile([16, NWIN * 64], f32, tag="ps")
                chunks = ((0, 512), (512, nw * 64)) if nw == 9 else ((0, 512),)
                for n0, n1 in chunks:
                    nc.tensor.matmul(ps[:, n0:n1], sel_t[:, :], rc[:, n0:n1],
                                     start=True, stop=True)
                vs = opool.tile([16, NWIN * 64], f32, tag="vs")
                nc.scalar.copy(vs[:, :nw * 64], ps[:, :nw * 64])
                nc.sync.dma_start(
                    vol[bass.ds((u * 17 + dyi) * 16, 16), 0:nw * 64],
                    vs[:, :nw * 64])

        if static:
            if repeat > 1:
                with tc.For_i(0, repeat, 1) as _r:
                    for u in range(nproc):
                        unit_body(u)
            else:
                for u in range(nproc):
                    unit_body(u)
        else:
            with tc.For_i(0, nproc, 1) as u:
                unit_body(u)

    nc.compile()
    return nc


def _get_nc():
    global _CACHED_NC
    if _CACHED_NC is None:
        _CACHED_NC = _build_nc(UPC, static=True)
    return _CACHED_NC


def _shift_mats():
    """0/1 shift matrices: Ish[p,:] = I_unit[p + ny + 8, :] (rows beyond the
    unit's 144 loaded rows are zero-filled via absent selector bits; image
    boundary rows are zero in xI itself)."""
    sall = np.zeros((128, 17 * 128), np.float32)
    shi = np.zeros((16, 17 * 128), np.float32)
    for dyi in range(17):
        ny = dyi - 8
        for p in range(128):
            k = p + ny + 8
            if 0 <= k < 128:
                sall[k, dyi * 128 + p] = 1.0
            elif 128 <= k < 144:
                shi[k - 128, dyi * 128 + p] = 1.0
    return sall, shi


def _unit_list():
    return [(b, t, c) for b in range(B) for t in range(T - 1)
            for c in range(CHUNKS)]


def _pack_inputs(vids):
    """Per-core xP/xI buffers.  vids: (B, T, 512, 512) f32."""
    units = _unit_list()
    sall, shi = _shift_mats()
    sel = (np.arange(128)[:, None] // 8 == np.arange(16)[None, :])
    sel = np.ascontiguousarray(sel, np.float32)
    in_maps = []
    assign = []
    for k in range(NCORES):
        mine = units[k::NCORES]
        assign.append(mine)
        xP = np.zeros((UPC, 128, 512), np.float32)
        xI = np.zeros((UPC, 144, WI), np.float32)
        for i, (b, t, c) in enumerate(mine):
            r0 = c * 128
            xP[i] = vids[b, t + 1, r0:r0 + 128, :]
            ir0 = r0 - 8
            lo, hi = max(ir0, 0), min(ir0 + 144, H)
            xI[i, lo - ir0:hi - ir0, 8:520] = vids[b, t, lo:hi, :]
        in_maps.append({"xP": xP.reshape(UPC * 128, 512),
                        "xI": xI.reshape(UPC * 144, WI),
                        "sall": sall, "shi": shi, "sel": sel})
    return in_maps, assign


def _assemble_vols(results, assign):
    """-> vol (NPAIR, 64, 64, 17, 17) f32 cost sums; only even-parity
    (ny+nx) entries are populated (garbage elsewhere / where invalid)."""
    vol = np.empty((NPAIR, NBR, NBC, 17, 17), np.float32)
    for k in range(NCORES):
        out = np.asarray(results[k]["vol"]).reshape(UPC, 17, 16, NWIN, 64)
        for i, (b, t, c) in enumerate(assign[k]):
            n = b * (T - 1) + t
            for dyi in range(17):
                parity = dyi & 1
                nj = 8 if parity else 9
                # out[i,dyi]: (16bi, 9dx, 64bj) -> (bi, bj, dx)
                blk = out[i, dyi, :, :nj].transpose(0, 2, 1)
                vol[n, 16 * c:16 * c + 16, :, dyi,
                    parity:parity + 2 * nj:2] = blk
    return vol


def _valid(bi, bj, ny, nx):
    y = bi * MB + ny
    x = bj * MB + nx
    return ((np.abs(ny) <= P) & (np.abs(nx) <= P)
            & (y >= 0) & (y + MB <= H) & (x >= 0) & (x + MB <= W))


def _sdsp_costs(slide, pblk, bi, bj, pair, cy, cx, k):
    """Host SAD cost sums at SDSP candidate k (odd parity), LARGE if
    invalid.  Summation order matches the device (sequential over the 8
    columns, then sequential over the 8 rows)."""
    ny = cy + SDSP[k, 1]
    nx = cx + SDSP[k, 0]
    ok = _valid(bi, bj, ny, nx)
    ys = np.clip(bi * MB + ny, 0, H - MB)
    xs = np.clip(bj * MB + nx, 0, W - MB)
    refw = slide[pair, ys, xs]                        # (N, 8, 8)
    d = np.abs(pblk - refw)
    cs = d[:, :, 0].copy()
    for i in range(1, 8):
        cs += d[:, :, i]
    r = cs[:, 0].copy()
    for i in range(1, 8):
        r += cs[:, i]
    return np.where(ok, r, LARGE_SUM)


def _walk(vol, vids):
    """Diamond search on even-parity cost-sum tables + host SDSP.
    Returns motion (NPAIR, 64, 64, 2) int32 as (dy, dx)."""
    N = NPAIR * NBR * NBC
    v = vol.reshape(N, 17, 17)
    bi = np.tile(np.repeat(np.arange(NBR), NBC), NPAIR)
    bj = np.tile(np.arange(NBC), NPAIR * NBR)
    cy = np.zeros(N, np.int32)
    cx = np.zeros(N, np.int32)
    done = v[:, 8, 8] == 0.0
    rows = np.arange(N)
    for _ in range(MAX_STEPS):
        ny = cy[:, None] + LDSP[None, :, 1]
        nx = cx[:, None] + LDSP[None, :, 0]
        ok = _valid(bi[:, None], bj[:, None], ny, nx)
        c = v[rows[:, None], np.clip(ny, -8, 8) + 8, np.clip(nx, -8, 8) + 8]
        c = np.where(ok, c, LARGE_SUM)
        pt = np.argmin(c, axis=1)
        move = ~done
        cy = np.where(move, cy + LDSP[pt, 1], cy)
        cx = np.where(move, cx + LDSP[pt, 0], cx)
        done |= pt == 4
        if done.all():
            break

    # SDSP refinement: center cost from the device volume; the 4 odd-parity
    # neighbours computed on host (data-dependent positions)
    pair = np.repeat(np.arange(NPAIR), NBR * NBC)
    pblk = vids[:, 1:].reshape(B, T - 1, NBR, MB, NBC, MB)
    pblk = pblk.transpose(0, 1, 2, 4, 3, 5).reshape(N, MB, MB)
    slide = np.lib.stride_tricks.sliding_window_view(
        np.ascontiguousarray(vids[:, :T - 1].reshape(NPAIR, H, W)),
        (MB, MB), axis=(1, 2))
    cands = np.empty((N, 5), np.float32)
    cands[:, 2] = v[rows, np.clip(cy, -8, 8) + 8, np.clip(cx, -8, 8) + 8]
    for k in (0, 1, 3, 4):
        cands[:, k] = _sdsp_costs(slide, pblk, bi, bj, pair, cy, cx, k)
    spt = np.argmin(cands, axis=1)
    cy = cy + SDSP[spt, 1]
    cx = cx + SDSP[spt, 0]
    return np.stack([cy, cx], -1).reshape(NPAIR, NBR, NBC, 2)


def _compensate(vids, motion):
    """pred frames: warp vids[b, t+1] by motion[b*(T-1)+t] for t in 0..T-3."""
    TT = T - 2
    b_idx = np.arange(B)[:, None, None, None]
    t_idx = np.arange(TT)[None, :, None, None]
    m = motion.reshape(B, T - 1, NBR, NBC, 2)[:, :TT]
    ys = np.arange(NBR)[None, None, :, None] * MB + m[:, :, :, :, 0]
    xs = np.arange(NBC)[None, None, None, :] * MB + m[:, :, :, :, 1]
    rows = ys[..., None, None] + np.arange(MB)[None, None, None, None, :, None]
    cols = xs[..., None, None] + np.arange(MB)[None, None, None, None, None, :]
    src = vids[:, 1:T - 1]
    blocks = src[b_idx[..., None, None], t_idx[..., None, None], rows, cols]
    return blocks.transpose(0, 1, 2, 4, 3, 5).reshape(B, TT, H, W)


def kernel(x):
    x = np.ascontiguousarray(np.asarray(x), dtype=np.float32)
    vids = x[:, 0]
    in_maps, assign = _pack_inputs(vids)
    nc = _get_nc()
    res = run_bass_kernel_spmd(nc, in_maps, core_ids=list(range(NCORES)))
    vol = _assemble_vols(res.results, assign)
    motion = _walk(vol, vids)
    pred = _compensate(vids, motion)[:, :, CROP:-CROP, CROP:-CROP]
    target = vids[:, 2:, CROP:-CROP, CROP:-CROP]
    return target[:, None].copy(), pred[:, None].copy()


if __name__ == "__main__":
    x = np.load("/tmp/x_input.npy")
    t, p = kernel(x)
    et = np.load("/tmp/exp_target.npy")
    ep = np.load("/tmp/exp_pred.npy")
    print("target equal:", np.array_equal(t, et))
    print("pred equal:", np.array_equal(p, ep))
    d = p - ep
    print("n diff:", int((d != 0).sum()), "rel:",
          float(np.linalg.norm(d.ravel()) / np.linalg.norm(ep.ravel())))
